# revision 43
# baseline (speedup 1.0000x reference)
"""Causal multi-head self-attention on 8 Trainium2 NeuronCores.

Sharding: tensor-parallel over heads. Each of the 8 cores owns 2 heads
(128 of the 1024 qkv dims). All matmuls bf16 (fp32 PSUM accumulate).
out_partial = A_c^T @ Wo_c^T per core; host sums the 8 partials.

Schedule: attention is a per-q-chunk (QCH=512) k-tile loop,
software-pipelined scores -> exp -> attn@V with the PE kept dense by a
filler queue (next batch's projections, previous chunk's finish work).

PE-centric design notes:
  - causal mask applied by an accumulating matmul (ident^T @ maskneg)
    into the scores PSUM, so exp/attnV never wait on the Vector engine;
  - attn@V carries a ones-column per head: the matmul also emits the
    softmax denominator (row 64 of the att PSUM);
  - the att PSUM is single-buffered but released right away by one au
    evacuation; the denominator row is spread to all 128 partitions by
    a C=1 ones-stationary matmul per head, reciprocated in one
    custom-DVE reciprocal_approx_fast, and the normalize multiplies
    run on the (otherwise idle) GpSimd engine from SBUF;
  - V is projected directly into [token, dim] layout (x-tile
    stationary, wv moving), so no PE transposes;
  - hardware pitfall (measured): two concurrently-draining row-tiled
    matmuls must NOT share a PSUM bank (device wedge). At QCH=512 the
    two head planes of scores/att tiles land in separate banks.

PSUM budget (8 banks): scores 2 x [128,2,512](2 banks) + att
1 x [65,2,512](2) + general 2 x [128,512](1).
"""

import numpy as np
from contextlib import ExitStack

import concourse.bass as bass
import concourse.mybir as mybir
import concourse.tile as tile
from concourse import bacc

F32 = mybir.dt.float32
BF16 = mybir.dt.bfloat16
EXP = mybir.ActivationFunctionType.Exp
COPY = mybir.ActivationFunctionType.Copy
MULT = mybir.AluOpType.mult


class Cfg:
    def __init__(self, B=4, S=2048, D=1024, TCH=512, QCH=512, mm_dt="bf16"):
        self.B, self.S, self.D = B, S, D
        self.T = B * S
        self.KT = D // 128          # contraction tiles for projections
        self.TCH = TCH              # token chunk for projections
        self.QCH = QCH              # query chunk for attention
        self.NQC = S // QCH         # q chunks per batch
        self.HD = 64
        self.mm_dt = "bf16"         # bf16 only
        assert S % QCH == 0 and QCH % 128 == 0 and self.T % TCH == 0


class Emitter:
    """Drains filler closures into attention k-tile slots.

    clock = k-tile iterations emitted so far; each filler has a
    ready-stamp (don't emit before this clock) and a cost in slots
    (throttles how much work lands per slot). chain keys give strict
    FIFO order within a chain and at most one unit per chain per tick.
    """

    def __init__(self, cap=4.0):
        self.clock = 0
        self.credit = 0.0
        self.cap = cap
        self.fillers = []           # [ready, cost, fn, chain]

    def push(self, ready, cost, fn, chain=None):
        self.fillers.append([ready, cost, fn, chain])

    def tick(self):
        self.clock += 1
        self.credit = min(self.credit + 1.0, self.cap)
        while True:
            idx = None
            seen = set()
            for j, f in enumerate(self.fillers):
                ok = f[0] <= self.clock and f[1] <= self.credit
                if ok and (f[3] is None or f[3] not in seen):
                    idx = j
                    break
                if f[3] is not None:
                    seen.add(f[3])
            if idx is None:
                break
            f = self.fillers.pop(idx)
            self.credit -= f[1]
            f[2]()

    def drain(self, chain):
        """Emit all remaining fillers of one chain, FIFO, ignoring
        stamps/credit. Needed as a barrier: a filler writing data that
        upcoming inline code reads MUST be emitted first (Tile tracks
        deps in program order)."""
        rest = []
        for f in self.fillers:
            if f[3] == chain:
                f[2]()
            else:
                rest.append(f)
        self.fillers = rest

    def flush(self):
        for f in self.fillers:
            f[2]()
        self.fillers = []


def build_program(cfg: Cfg):
    """Build the SPMD single-core Bass program (same program all cores)."""
    nc = bacc.Bacc("TRN2", target_bir_lowering=False, debug=False)
    B, S, D, T, KT = cfg.B, cfg.S, cfg.D, cfg.T, cfg.KT
    TCH, QCH, NQC = cfg.TCH, cfg.QCH, cfg.NQC
    NVT = T // 128                 # number of 128-token V tiles
    NCH = S // TCH                 # projection chunks per batch
    OC = 512                       # out-proj column tile

    xT_d = nc.dram_tensor("xT", [128, KT, T], BF16, kind="ExternalInput")
    wq_d = nc.dram_tensor("wq", [128, KT, 128], BF16, kind="ExternalInput")
    wk_d = nc.dram_tensor("wk", [128, KT, 128], BF16, kind="ExternalInput")
    wv_d = nc.dram_tensor("wv", [128, KT, 128], BF16, kind="ExternalInput")
    wo_d = nc.dram_tensor("wo", [128, D], BF16, kind="ExternalInput")
    mneg_d = nc.dram_tensor("maskneg", [128, 128], BF16, kind="ExternalInput")
    ident_d = nc.dram_tensor("ident", [128, 128], BF16, kind="ExternalInput")
    out_d = nc.dram_tensor("out_p", [T, D], BF16, kind="ExternalOutput")
    out_r = out_d.rearrange("(n p) o -> p n o", p=128)   # [128, NVT, D]

    with tile.TileContext(nc) as tc, ExitStack() as ctx:
        persist = ctx.enter_context(tc.tile_pool(name="persist", bufs=1))

        qt_sb = persist.tile([128, T], BF16, tag="qt")
        kt_sb = persist.tile([128, T], BF16, tag="kt")
        a_sb = persist.tile([128, T], BF16, tag="a")
        # V natural layout + one ones-column per head (attn@V emits the
        # softmax denominator in its row 64):
        #   cols 0:64 = head0 dims, col 64 = 1.0,
        #   cols 65:129 = head1 dims, col 129 = 1.0
        v_sb = persist.tile([128, NVT, 130], BF16, tag="v")
        wq_sb = persist.tile([128, KT, 128], BF16, tag="wq")
        wk_sb = persist.tile([128, KT, 128], BF16, tag="wk")
        wv_sb = persist.tile([128, KT, 128], BF16, tag="wv")
        wo_sb = persist.tile([128, D], BF16, tag="wo")
        mneg_sb = persist.tile([128, 128], BF16, tag="mneg")
        ident = persist.tile([128, 128], BF16, tag="ident")
        onesr = persist.tile([65, 128], BF16, tag="onesr")
        ones128 = persist.tile([128, 1], F32, tag="ones128")

        xp = ctx.enter_context(tc.tile_pool(name="xp", bufs=8))
        ptp = ctx.enter_context(tc.tile_pool(name="ptp", bufs=5))
        aup = ctx.enter_context(tc.tile_pool(name="aup", bufs=2))
        rsp_s = ctx.enter_context(tc.tile_pool(name="rsps", bufs=2))
        op = ctx.enter_context(tc.tile_pool(name="op", bufs=4))
        # PSUM: 8 banks. scores [128,2,512] f32 puts the two head planes
        # in SEPARATE banks (concurrently-draining row-tiled matmuls
        # into one bank wedge the device). att [65,2,512] likewise; gpp
        # serves projections, out-proj and the reciprocal spreads.
        scp = ctx.enter_context(tc.tile_pool(name="scp", bufs=2, space="PSUM"))
        attp = ctx.enter_context(tc.tile_pool(name="attp", bufs=1,
                                              space="PSUM"))
        gpp = ctx.enter_context(tc.tile_pool(name="gpp", bufs=2, space="PSUM"))

        em = Emitter()

        # ---------------- projections ----------------
        def x_dma(b, tci, first=False):
            x_t = xp.tile([128, KT, TCH], BF16, tag="x")
            t0 = b * S + tci * TCH
            nsplit = 4 if first else 2
            step = KT // nsplit
            for si in range(nsplit):
                nc.sync.dma_start(
                    x_t[:, si * step:(si + 1) * step, :],
                    xT_d[:, si * step:(si + 1) * step, t0:t0 + TCH])
            return x_t

        def proj_chunk_units(b, tci, hold):
            """Filler sub-units projecting chunk tci of batch b into
            qt/kt/v. Every unit's PSUM use is self-contained (alloc,
            matmuls, evacuation), so interleaved chains stay safe."""
            t0 = b * S + tci * TCH
            units = []

            def qk_chain(w_sb, dst, eng):
                x_t = hold["x"]
                ps = gpp.tile([128, TCH], F32, tag="gp")
                for kt in range(KT):
                    nc.tensor.matmul(
                        ps[:], w_sb[:, kt, :], x_t[:, kt, :],
                        start=(kt == 0), stop=(kt == KT - 1))
                if eng == "act":      # balance bank-release across queues
                    nc.scalar.activation(dst[:, t0:t0 + TCH], ps[:], COPY)
                else:
                    nc.vector.tensor_copy(dst[:, t0:t0 + TCH], ps[:])

            def v_tile(tl):
                x_t = hold["x"]
                ps = gpp.tile([128, TCH], F32, tag="gp")
                pv = ps[:, 0:128]
                for kt in range(KT):
                    nc.tensor.matmul(
                        pv, x_t[:, kt, tl * 128:(tl + 1) * 128],
                        wv_sb[:, kt, :],
                        start=(kt == 0), stop=(kt == KT - 1))
                vt = (t0 + tl * 128) // 128
                if tl % 2 == 0:
                    nc.vector.tensor_copy(v_sb[:, vt, 0:64], pv[:, 0:64])
                    nc.vector.tensor_copy(v_sb[:, vt, 65:129], pv[:, 64:128])
                else:
                    nc.scalar.activation(v_sb[:, vt, 0:64], pv[:, 0:64], COPY)
                    nc.scalar.activation(v_sb[:, vt, 65:129], pv[:, 64:128],
                                         COPY)

            units.append((1.5, lambda: qk_chain(wq_sb, qt_sb, "act")))
            units.append((1.5, lambda: qk_chain(wk_sb, kt_sb, "dve")))
            for tl in range(TCH // 128):
                units.append((0.8, lambda tl=tl: v_tile(tl)))
            return units

        def push_proj(b, ready_by_tci=None, inline_first_dma=False):
            # per-chunk ready stamps let late chunks pop inside batch
            # b's own attention; x DMAs prefetch during the previous one
            for tci in range(NCH):
                hold = {}
                chain = ("proj", b, tci)

                def dma_u(b=b, tci=tci, hold=hold):
                    hold["x"] = x_dma(b, tci, first=(b == 0 and tci == 0))

                if inline_first_dma and tci == 0:
                    dma_u()
                else:
                    em.push(0, 0.0, dma_u, chain=chain)
                rdy = ready_by_tci[tci] if ready_by_tci else 0
                for cost, fn in proj_chunk_units(b, tci, hold):
                    em.push(rdy, cost, fn, chain=chain)

        # ---------------- attention ----------------
        fin_pending = []            # [(global_chunk_idx, chain_key)]

        def push_finish(b, qc, g, att):
            """Finish pipeline for chunk (b, qc): au evacuation (frees
            the single att PSUM buf) -> per-head denominator spread ->
            fast reciprocal -> GpSimd normalize -> out-proj."""
            base = b * S
            q0 = qc * QCH
            cols = slice(base + q0, base + q0 + QCH)
            t_base = (base + q0) // 128
            chain = ("fin", g)
            hold = {}
            last = (g == B * NQC - 1)

            def warm_mm():
                # dependency-free matmul: keeps the HAM clock gate open
                # through the final chunk's serial finish chain
                ps = gpp.tile([128, OC], F32, tag="gp")
                nc.tensor.matmul(ps[:], wo_sb[:, 0:128], wo_sb[:, 0:OC],
                                 start=True, stop=True)

            def u_evac():
                au = aup.tile([65, 2, QCH], BF16, tag="au")
                nc.vector.tensor_copy(au[:], att[:])
                hold["au"] = au

            def u_rh():
                if last:
                    warm_mm()
                au = hold["au"]
                rsps = rsp_s.tile([64, 2, QCH], F32, tag="rsps")
                for h in (0, 1):
                    rsp = gpp.tile([128, QCH], F32, tag="gp")
                    nc.tensor.matmul(rsp[:], onesr[64:65, :],
                                     au[64:65, h, :], start=True, stop=True)
                    with nc.allow_low_precision(reason="fp32 approx recip"):
                        nc.vector.reciprocal_approx_fast(
                            rsps[:, h, :], rsp[0:64, :])
                hold["rsps"] = rsps

            def u_norm():
                if last:
                    warm_mm()
                    warm_mm()
                au = hold.pop("au")
                rsps = hold.pop("rsps")
                # SBUF-only multiplies on the GpSimd engine
                nc.gpsimd.tensor_tensor(
                    a_sb[0:64, cols], au[0:64, 0, :], rsps[:, 0, :], MULT)
                nc.gpsimd.tensor_tensor(
                    a_sb[64:128, cols], au[0:64, 1, :], rsps[:, 1, :], MULT)

            def u_oproj(ti):
                if last:
                    warm_mm()
                tt = t_base + ti
                o_sb = op.tile([128, D], BF16, tag="osb")
                for oc in range(D // OC):
                    ps = gpp.tile([128, OC], F32, tag="gp")
                    nc.tensor.matmul(
                        ps[:], a_sb[:, tt * 128:(tt + 1) * 128],
                        wo_sb[:, oc * OC:(oc + 1) * OC],
                        start=True, stop=True)
                    if oc == 0:
                        nc.vector.tensor_copy(o_sb[:, oc * OC:(oc + 1) * OC],
                                              ps[:])
                    else:
                        nc.scalar.activation(o_sb[:, oc * OC:(oc + 1) * OC],
                                             ps[:], COPY)
                nc.gpsimd.dma_start(out_r[:, tt, :], o_sb[:])

            now = em.clock
            em.push(now + 1, 0.8, u_evac, chain=chain)
            em.push(now + 1, 0.6, u_rh, chain=chain)
            em.push(now + 2, 0.3, u_norm, chain=chain)
            for ti in range(QCH // 128):
                em.push(now + 2 + ti, 1.0, lambda ti=ti: u_oproj(ti),
                        chain=chain)
            fin_pending.append((g, chain))

        def attn_chunk(b, qc):
            g = b * NQC + qc
            # emission barriers: ALL previous finish chains must be
            # emitted before this chunk (att PSUM is single-buffered);
            # this batch's projections must cover tokens up to the end
            # of this chunk before scores/attnV read qt/kt/v.
            for gg, ck in list(fin_pending):
                if gg <= g - 1:
                    em.drain(ck)
                    fin_pending.remove((gg, ck))
            # batch 0 prefetches one proj chunk ahead so the DVE
            # evacuations complete before scores need qt/kt
            need_tci = ((qc + 1) * QCH - 1) // TCH + (1 if b == 0 else 0)
            for tci in range(min(need_tci + 1, NCH)):
                em.drain(("proj", b, tci))

            base = b * S
            vbase = base // 128
            q0 = qc * QCH
            n_kt = (q0 + QCH) // 128
            att = attp.tile([65, 2, QCH], F32, tag="att")

            def attv(kti):
                k0 = kti * 128
                co = max(0, k0 - q0)
                pt = attv.pts.pop(kti)
                for h in (0, 1):
                    nc.tensor.matmul(
                        att[:, h, co:QCH],
                        v_sb[:, vbase + kti, h * 65:h * 65 + 65],
                        pt[:, h, co:QCH],
                        start=(kti == 0), stop=(kti == n_kt - 1),
                        skip_group_check=True)
            attv.pts = {}

            for kti in range(n_kt):
                k0 = kti * 128
                co = max(0, k0 - q0)
                diag = k0 >= q0
                sc = scp.tile([128, 2, QCH], F32, tag="sc")
                for h in (0, 1):
                    # 64-contraction pair packs onto PE row halves; the
                    # two head planes drain into separate PSUM banks
                    nc.tensor.matmul(
                        sc[:, h, co:QCH],
                        kt_sb[h * 64:(h + 1) * 64,
                              base + k0:base + k0 + 128],
                        qt_sb[h * 64:(h + 1) * 64,
                              base + q0 + co:base + q0 + QCH],
                        start=True, stop=(not diag),
                        skip_group_check=True)
                if diag:
                    # additive causal mask folded into the PE stream
                    for h in (0, 1):
                        nc.tensor.matmul(
                            sc[:, h, co:co + 128], ident[:], mneg_sb[:],
                            start=False, stop=True,
                            skip_group_check=True)
                pt = ptp.tile([128, 2, QCH], BF16, tag="pt")
                nc.scalar.activation(pt[:, :, co:QCH], sc[:, :, co:QCH], EXP)
                attv.pts[kti] = pt
                em.tick()
                # attn@V runs three slots behind scores so its exp
                # input has two slots of latency slack
                if kti >= 3:
                    attv(kti - 3)
            for kk in range(max(0, n_kt - 3), n_kt):
                attv(kk)
            push_finish(b, qc, g, att)

        # ---------------- emission ----------------
        # startup: only wq + the first x split gate the first GEMM chain
        nc.sync.dma_start(wq_sb[:], wq_d[:])
        push_proj(0, inline_first_dma=True)
        nc.sync.dma_start(wk_sb[:], wk_d[:])
        nc.sync.dma_start(wv_sb[:], wv_d[:])
        nc.sync.dma_start(ident[:], ident_d[:])
        nc.sync.dma_start(mneg_sb[:], mneg_d[:])
        nc.sync.dma_start(wo_sb[:], wo_d[:])
        nc.vector.memset(onesr[:], 1.0)
        nc.vector.memset(ones128[:], 1.0)
        nc.vector.tensor_copy(
            v_sb[:, :, 64:65],
            ones128[:, None, :].to_broadcast((128, NVT, 1)))
        nc.vector.tensor_copy(
            v_sb[:, :, 129:130],
            ones128[:, None, :].to_broadcast((128, NVT, 1)))

        for b in range(B):
            if b + 1 < B:
                if b + 1 == B - 1:
                    # keep filler work for the (otherwise exp-paced)
                    # last batch: its chunks 2-3 projections pop inside
                    # its own early attention slots
                    now = em.clock
                    push_proj(b + 1,
                              ready_by_tci=[0, 0, now + 40, now + 46])
                else:
                    push_proj(b + 1)
            for qc in range(NQC):
                attn_chunk(b, qc)
        em.flush()

    nc.compile()
    return nc


def prep_inputs(in_features, weight_q, weight_k, weight_v, weight_o, cfg: Cfg,
                n_cores=8):
    """Host-side shard/layout prep. Returns per-core input dicts."""
    import ml_dtypes
    mmnp = ml_dtypes.bfloat16
    B, S, D, T, KT = cfg.B, cfg.S, cfg.D, cfg.T, cfg.KT
    x = np.asarray(in_features, dtype=np.float32).reshape(T, D)
    # xT[p, kt, t] = x[t, kt*128 + p]
    xT = np.ascontiguousarray(
        x.T.reshape(KT, 128, T).transpose(1, 0, 2)).astype(mmnp)
    # additive causal mask for a 128x128 diagonal block: 0 keep, -60 kill
    maskneg = np.where(np.arange(128)[:, None] > np.arange(128)[None, :],
                       np.float32(-60.0), np.float32(0.0)).astype(mmnp)
    wq = np.asarray(weight_q, dtype=np.float32) * (1.0 / np.sqrt(cfg.HD))
    wk = np.asarray(weight_k, dtype=np.float32)
    wv = np.asarray(weight_v, dtype=np.float32)
    wo = np.asarray(weight_o, dtype=np.float32)

    def wslice(w, c):
        # [128, KT, 128]: ws[p, kt, m] = w[c*128 + m, kt*128 + p]
        ws = w[c * 128:(c + 1) * 128, :]                  # [128, D]
        return np.ascontiguousarray(
            ws.T.reshape(KT, 128, 128).transpose(1, 0, 2)).astype(mmnp)

    in_maps = []
    for c in range(n_cores):
        in_maps.append({
            "xT": xT,
            "wq": wslice(wq, c),
            "wk": wslice(wk, c),
            "wv": wslice(wv, c),
            "wo": np.ascontiguousarray(
                wo[:, c * 128:(c + 1) * 128].T).astype(mmnp),
            "maskneg": maskneg,
            "ident": np.eye(128, dtype=mmnp),
        })
    return in_maps


_CACHE = {}


def _get_program(cfg: Cfg):
    key = (cfg.B, cfg.S, cfg.D, cfg.TCH, cfg.QCH, cfg.mm_dt)
    if key not in _CACHE:
        _CACHE[key] = build_program(cfg)
    return _CACHE[key]


def run(inputs, cfg: Cfg, trace=False, trace_kwargs=None):
    import time
    from concourse.bass_utils import run_bass_kernel_spmd
    nc = _get_program(cfg)
    in_maps = prep_inputs(**inputs, cfg=cfg)
    last = None
    for attempt in range(3):
        try:
            res = run_bass_kernel_spmd(
                nc, in_maps, core_ids=list(range(8)), trace=trace,
                **(trace_kwargs or {}))
            break
        except Exception as e:  # transient NRT device wedges happen
            last = e
            time.sleep(10)
    else:
        raise last
    parts = [np.asarray(r["out_p"], dtype=np.float32) for r in res.results]
    out = np.sum(np.stack(parts, 0), axis=0)
    return out.astype(np.float32).reshape(cfg.B, cfg.S, cfg.D), res


def kernel(in_features, weight_q, weight_k, weight_v, weight_o):
    cfg = Cfg()
    out, _ = run(dict(in_features=in_features, weight_q=weight_q,
                      weight_k=weight_k, weight_v=weight_v,
                      weight_o=weight_o), cfg)
    return out


# revision 44
# speedup vs baseline: 1.0096x; 1.0096x over previous
"""Causal multi-head self-attention on 8 Trainium2 NeuronCores.

Sharding: tensor-parallel over heads. Each of the 8 cores owns 2 heads
(128 of the 1024 qkv dims). All matmuls bf16 (fp32 PSUM accumulate).
out_partial = A_c^T @ Wo_c^T per core; host sums the 8 partials.

Schedule: attention is a per-q-chunk (QCH=512) k-tile loop,
software-pipelined scores -> exp -> attn@V with the PE kept dense by a
filler queue (next batch's projections, previous chunk's finish work).

PE-centric design notes:
  - causal mask applied by an accumulating matmul (ident^T @ maskneg)
    into the scores PSUM, so exp/attnV never wait on the Vector engine;
  - attn@V carries a ones-column per head: the matmul also emits the
    softmax denominator (row 64 of the att PSUM);
  - the att PSUM is single-buffered but released right away by one au
    evacuation; the denominator row is spread to all 128 partitions by
    a C=1 ones-stationary matmul per head, reciprocated in one
    custom-DVE reciprocal_approx_fast, and the normalize multiplies
    run on the (otherwise idle) GpSimd engine from SBUF;
  - V is projected directly into [token, dim] layout (x-tile
    stationary, wv moving), so no PE transposes;
  - hardware pitfall (measured): two concurrently-draining row-tiled
    matmuls must NOT share a PSUM bank (device wedge). At QCH=512 the
    two head planes of scores/att tiles land in separate banks.

PSUM budget (8 banks): scores 2 x [128,2,512](2 banks) + att
1 x [65,2,512](2) + general 2 x [128,512](1).
"""

import numpy as np
from contextlib import ExitStack

import concourse.bass as bass
import concourse.mybir as mybir
import concourse.tile as tile
from concourse import bacc

F32 = mybir.dt.float32
BF16 = mybir.dt.bfloat16
EXP = mybir.ActivationFunctionType.Exp
COPY = mybir.ActivationFunctionType.Copy
MULT = mybir.AluOpType.mult


class Cfg:
    def __init__(self, B=4, S=2048, D=1024, TCH=512, QCH=512, mm_dt="bf16"):
        self.B, self.S, self.D = B, S, D
        self.T = B * S
        self.KT = D // 128          # contraction tiles for projections
        self.TCH = TCH              # token chunk for projections
        self.QCH = QCH              # query chunk for attention
        self.NQC = S // QCH         # q chunks per batch
        self.HD = 64
        self.mm_dt = "bf16"         # bf16 only
        assert S % QCH == 0 and QCH % 128 == 0 and self.T % TCH == 0


class Emitter:
    """Drains filler closures into attention k-tile slots.

    clock = k-tile iterations emitted so far; each filler has a
    ready-stamp (don't emit before this clock) and a cost in slots
    (throttles how much work lands per slot). chain keys give strict
    FIFO order within a chain and at most one unit per chain per tick.
    """

    def __init__(self, cap=4.0):
        self.clock = 0
        self.credit = 0.0
        self.cap = cap
        self.fillers = []           # [ready, cost, fn, chain]

    def push(self, ready, cost, fn, chain=None):
        self.fillers.append([ready, cost, fn, chain])

    def tick(self):
        self.clock += 1
        self.credit = min(self.credit + 1.0, self.cap)
        while True:
            idx = None
            seen = set()
            for j, f in enumerate(self.fillers):
                ok = f[0] <= self.clock and f[1] <= self.credit
                if ok and (f[3] is None or f[3] not in seen):
                    idx = j
                    break
                if f[3] is not None:
                    seen.add(f[3])
            if idx is None:
                break
            f = self.fillers.pop(idx)
            self.credit -= f[1]
            f[2]()

    def drain(self, chain):
        """Emit all remaining fillers of one chain, FIFO, ignoring
        stamps/credit. Needed as a barrier: a filler writing data that
        upcoming inline code reads MUST be emitted first (Tile tracks
        deps in program order)."""
        rest = []
        for f in self.fillers:
            if f[3] == chain:
                f[2]()
            else:
                rest.append(f)
        self.fillers = rest

    def flush(self):
        for f in self.fillers:
            f[2]()
        self.fillers = []


def build_program(cfg: Cfg):
    """Build the SPMD single-core Bass program (same program all cores)."""
    nc = bacc.Bacc("TRN2", target_bir_lowering=False, debug=False)
    B, S, D, T, KT = cfg.B, cfg.S, cfg.D, cfg.T, cfg.KT
    TCH, QCH, NQC = cfg.TCH, cfg.QCH, cfg.NQC
    NVT = T // 128                 # number of 128-token V tiles
    NCH = S // TCH                 # projection chunks per batch
    OC = 512                       # out-proj column tile

    xT_d = nc.dram_tensor("xT", [128, KT, T], BF16, kind="ExternalInput")
    wq_d = nc.dram_tensor("wq", [128, KT, 128], BF16, kind="ExternalInput")
    wk_d = nc.dram_tensor("wk", [128, KT, 128], BF16, kind="ExternalInput")
    wv_d = nc.dram_tensor("wv", [128, KT, 128], BF16, kind="ExternalInput")
    wo_d = nc.dram_tensor("wo", [128, D], BF16, kind="ExternalInput")
    mneg_d = nc.dram_tensor("maskneg", [128, 128], BF16, kind="ExternalInput")
    ident_d = nc.dram_tensor("ident", [128, 128], BF16, kind="ExternalInput")
    out_d = nc.dram_tensor("out_p", [T, D], BF16, kind="ExternalOutput")
    out_r = out_d.rearrange("(n p) o -> p n o", p=128)   # [128, NVT, D]

    with tile.TileContext(nc) as tc, ExitStack() as ctx:
        persist = ctx.enter_context(tc.tile_pool(name="persist", bufs=1))

        qt_sb = persist.tile([128, T], BF16, tag="qt")
        kt_sb = persist.tile([128, T], BF16, tag="kt")
        a_sb = persist.tile([128, T], BF16, tag="a")
        # V natural layout + one ones-column per head (attn@V emits the
        # softmax denominator in its row 64):
        #   cols 0:64 = head0 dims, col 64 = 1.0,
        #   cols 65:129 = head1 dims, col 129 = 1.0
        v_sb = persist.tile([128, NVT, 130], BF16, tag="v")
        wq_sb = persist.tile([128, KT, 128], BF16, tag="wq")
        wk_sb = persist.tile([128, KT, 128], BF16, tag="wk")
        wv_sb = persist.tile([128, KT, 128], BF16, tag="wv")
        wo_sb = persist.tile([128, D], BF16, tag="wo")
        mneg_sb = persist.tile([128, 128], BF16, tag="mneg")
        ident = persist.tile([128, 128], BF16, tag="ident")
        onesr = persist.tile([65, 128], BF16, tag="onesr")
        ones128 = persist.tile([128, 1], F32, tag="ones128")

        xp = ctx.enter_context(tc.tile_pool(name="xp", bufs=8))
        ptp = ctx.enter_context(tc.tile_pool(name="ptp", bufs=5))
        aup = ctx.enter_context(tc.tile_pool(name="aup", bufs=3))
        rsp_s = ctx.enter_context(tc.tile_pool(name="rsps", bufs=2))
        op = ctx.enter_context(tc.tile_pool(name="op", bufs=4))
        # PSUM: 8 banks. scores [128,2,512] f32 puts the two head planes
        # in SEPARATE banks (concurrently-draining row-tiled matmuls
        # into one bank wedge the device). att [65,2,512] likewise; gpp
        # serves projections, out-proj and the reciprocal spreads.
        scp = ctx.enter_context(tc.tile_pool(name="scp", bufs=2, space="PSUM"))
        attp = ctx.enter_context(tc.tile_pool(name="attp", bufs=1,
                                              space="PSUM"))
        gpp = ctx.enter_context(tc.tile_pool(name="gpp", bufs=2, space="PSUM"))

        em = Emitter()

        # ---------------- projections ----------------
        def x_dma(b, tci, first=False):
            x_t = xp.tile([128, KT, TCH], BF16, tag="x")
            t0 = b * S + tci * TCH
            nsplit = 4 if first else 2
            step = KT // nsplit
            for si in range(nsplit):
                nc.sync.dma_start(
                    x_t[:, si * step:(si + 1) * step, :],
                    xT_d[:, si * step:(si + 1) * step, t0:t0 + TCH])
            return x_t

        def proj_chunk_units(b, tci, hold):
            """Filler sub-units projecting chunk tci of batch b into
            qt/kt/v. Every unit's PSUM use is self-contained (alloc,
            matmuls, evacuation), so interleaved chains stay safe."""
            t0 = b * S + tci * TCH
            units = []

            def qk_chain(w_sb, dst, eng):
                x_t = hold["x"]
                ps = gpp.tile([128, TCH], F32, tag="gp")
                for kt in range(KT):
                    nc.tensor.matmul(
                        ps[:], w_sb[:, kt, :], x_t[:, kt, :],
                        start=(kt == 0), stop=(kt == KT - 1))
                if eng == "act":      # balance bank-release across queues
                    nc.scalar.activation(dst[:, t0:t0 + TCH], ps[:], COPY)
                else:
                    nc.vector.tensor_copy(dst[:, t0:t0 + TCH], ps[:])

            def v_tile(tl):
                x_t = hold["x"]
                ps = gpp.tile([128, TCH], F32, tag="gp")
                pv = ps[:, 0:128]
                for kt in range(KT):
                    nc.tensor.matmul(
                        pv, x_t[:, kt, tl * 128:(tl + 1) * 128],
                        wv_sb[:, kt, :],
                        start=(kt == 0), stop=(kt == KT - 1))
                vt = (t0 + tl * 128) // 128
                if tl % 2 == 0:
                    nc.vector.tensor_copy(v_sb[:, vt, 0:64], pv[:, 0:64])
                    nc.vector.tensor_copy(v_sb[:, vt, 65:129], pv[:, 64:128])
                else:
                    nc.scalar.activation(v_sb[:, vt, 0:64], pv[:, 0:64], COPY)
                    nc.scalar.activation(v_sb[:, vt, 65:129], pv[:, 64:128],
                                         COPY)

            units.append((1.5, lambda: qk_chain(wq_sb, qt_sb, "act")))
            units.append((1.5, lambda: qk_chain(wk_sb, kt_sb, "dve")))
            for tl in range(TCH // 128):
                units.append((0.8, lambda tl=tl: v_tile(tl)))
            return units

        def push_proj(b, ready_by_tci=None, inline_first_dma=False):
            # per-chunk ready stamps let late chunks pop inside batch
            # b's own attention; x DMAs prefetch during the previous one
            for tci in range(NCH):
                hold = {}
                chain = ("proj", b, tci)

                def dma_u(b=b, tci=tci, hold=hold):
                    hold["x"] = x_dma(b, tci, first=(b == 0 and tci == 0))

                if inline_first_dma and tci == 0:
                    dma_u()
                else:
                    em.push(0, 0.0, dma_u, chain=chain)
                rdy = ready_by_tci[tci] if ready_by_tci else 0
                for cost, fn in proj_chunk_units(b, tci, hold):
                    em.push(rdy, cost, fn, chain=chain)

        # ---------------- attention ----------------
        fin_pending = []            # [(global_chunk_idx, chain_key)]

        def push_finish(b, qc, g, att):
            """Finish pipeline for chunk (b, qc): au evacuation (frees
            the single att PSUM buf) -> per-head denominator spread ->
            fast reciprocal -> GpSimd normalize -> out-proj."""
            base = b * S
            q0 = qc * QCH
            cols = slice(base + q0, base + q0 + QCH)
            t_base = (base + q0) // 128
            chain = ("fin", g)
            hold = {}
            last = (g == B * NQC - 1)

            def warm_mm():
                # dependency-free matmul: keeps the HAM clock gate open
                # through the final chunk's serial finish chain
                ps = gpp.tile([128, OC], F32, tag="gp")
                nc.tensor.matmul(ps[:], wo_sb[:, 0:128], wo_sb[:, 0:OC],
                                 start=True, stop=True)

            def u_evac():
                au = aup.tile([65, 2, QCH], BF16, tag="au")
                nc.vector.tensor_copy(au[:], att[:])
                hold["au"] = au

            def u_rh():
                if last:
                    warm_mm()
                au = hold["au"]
                rsps = rsp_s.tile([64, 2, QCH], F32, tag="rsps")
                for h in (0, 1):
                    rsp = gpp.tile([128, QCH], F32, tag="gp")
                    nc.tensor.matmul(rsp[:], onesr[64:65, :],
                                     au[64:65, h, :], start=True, stop=True)
                    with nc.allow_low_precision(reason="fp32 approx recip"):
                        nc.vector.reciprocal_approx_fast(
                            rsps[:, h, :], rsp[0:64, :])
                hold["rsps"] = rsps

            def u_norm():
                if last:
                    warm_mm()
                    warm_mm()
                au = hold.pop("au")
                rsps = hold.pop("rsps")
                # SBUF-only multiplies on the GpSimd engine
                nc.gpsimd.tensor_tensor(
                    a_sb[0:64, cols], au[0:64, 0, :], rsps[:, 0, :], MULT)
                nc.gpsimd.tensor_tensor(
                    a_sb[64:128, cols], au[0:64, 1, :], rsps[:, 1, :], MULT)

            def u_oproj(ti):
                if last:
                    warm_mm()
                tt = t_base + ti
                o_sb = op.tile([128, D], BF16, tag="osb")
                for oc in range(D // OC):
                    ps = gpp.tile([128, OC], F32, tag="gp")
                    nc.tensor.matmul(
                        ps[:], a_sb[:, tt * 128:(tt + 1) * 128],
                        wo_sb[:, oc * OC:(oc + 1) * OC],
                        start=True, stop=True)
                    if oc == 0:
                        nc.vector.tensor_copy(o_sb[:, oc * OC:(oc + 1) * OC],
                                              ps[:])
                    else:
                        nc.scalar.activation(o_sb[:, oc * OC:(oc + 1) * OC],
                                             ps[:], COPY)
                # sync queue (mostly idle): keeps norms unblocked on
                # the GpSimd queue
                nc.sync.dma_start(out_r[:, tt, :], o_sb[:])

            now = em.clock
            em.push(now + 1, 0.8, u_evac, chain=chain)
            em.push(now + 1, 0.6, u_rh, chain=chain)
            em.push(now + 2, 0.3, u_norm, chain=chain)
            for ti in range(QCH // 128):
                em.push(now + 2 + ti, 1.0, lambda ti=ti: u_oproj(ti),
                        chain=chain)
            fin_pending.append((g, chain))

        def attn_chunk(b, qc):
            g = b * NQC + qc
            # emission barriers: ALL previous finish chains must be
            # emitted before this chunk (att PSUM is single-buffered);
            # this batch's projections must cover tokens up to the end
            # of this chunk before scores/attnV read qt/kt/v.
            for gg, ck in list(fin_pending):
                if gg <= g - 1:
                    em.drain(ck)
                    fin_pending.remove((gg, ck))
            # batch 0 prefetches one proj chunk ahead so the DVE
            # evacuations complete before scores need qt/kt
            need_tci = ((qc + 1) * QCH - 1) // TCH + (1 if b == 0 else 0)
            for tci in range(min(need_tci + 1, NCH)):
                em.drain(("proj", b, tci))

            base = b * S
            vbase = base // 128
            q0 = qc * QCH
            n_kt = (q0 + QCH) // 128
            att = attp.tile([65, 2, QCH], F32, tag="att")

            def attv(kti):
                k0 = kti * 128
                co = max(0, k0 - q0)
                pt = attv.pts.pop(kti)
                for h in (0, 1):
                    nc.tensor.matmul(
                        att[:, h, co:QCH],
                        v_sb[:, vbase + kti, h * 65:h * 65 + 65],
                        pt[:, h, co:QCH],
                        start=(kti == 0), stop=(kti == n_kt - 1),
                        skip_group_check=True)
            attv.pts = {}

            for kti in range(n_kt):
                k0 = kti * 128
                co = max(0, k0 - q0)
                diag = k0 >= q0
                sc = scp.tile([128, 2, QCH], F32, tag="sc")
                for h in (0, 1):
                    # 64-contraction pair packs onto PE row halves; the
                    # two head planes drain into separate PSUM banks
                    nc.tensor.matmul(
                        sc[:, h, co:QCH],
                        kt_sb[h * 64:(h + 1) * 64,
                              base + k0:base + k0 + 128],
                        qt_sb[h * 64:(h + 1) * 64,
                              base + q0 + co:base + q0 + QCH],
                        start=True, stop=(not diag),
                        skip_group_check=True)
                if diag:
                    # additive causal mask folded into the PE stream
                    for h in (0, 1):
                        nc.tensor.matmul(
                            sc[:, h, co:co + 128], ident[:], mneg_sb[:],
                            start=False, stop=True,
                            skip_group_check=True)
                pt = ptp.tile([128, 2, QCH], BF16, tag="pt")
                nc.scalar.activation(pt[:, :, co:QCH], sc[:, :, co:QCH], EXP)
                attv.pts[kti] = pt
                em.tick()
                # attn@V runs three slots behind scores so its exp
                # input has two slots of latency slack
                if kti >= 3:
                    attv(kti - 3)
            for kk in range(max(0, n_kt - 3), n_kt):
                attv(kk)
            push_finish(b, qc, g, att)

        # ---------------- emission ----------------
        # startup: only wq + the first x split gate the first GEMM chain
        nc.sync.dma_start(wq_sb[:], wq_d[:])
        push_proj(0, inline_first_dma=True)
        nc.sync.dma_start(wk_sb[:], wk_d[:])
        nc.sync.dma_start(wv_sb[:], wv_d[:])
        nc.sync.dma_start(ident[:], ident_d[:])
        nc.sync.dma_start(mneg_sb[:], mneg_d[:])
        nc.sync.dma_start(wo_sb[:], wo_d[:])
        nc.vector.memset(onesr[:], 1.0)
        nc.vector.memset(ones128[:], 1.0)
        nc.vector.tensor_copy(
            v_sb[:, :, 64:65],
            ones128[:, None, :].to_broadcast((128, NVT, 1)))
        nc.vector.tensor_copy(
            v_sb[:, :, 129:130],
            ones128[:, None, :].to_broadcast((128, NVT, 1)))

        for b in range(B):
            if b + 1 < B:
                if b + 1 == B - 1:
                    # keep filler work for the (otherwise exp-paced)
                    # last batch: its chunks 2-3 projections pop inside
                    # its own early attention slots
                    now = em.clock
                    push_proj(b + 1,
                              ready_by_tci=[0, 0, now + 40, now + 46])
                else:
                    push_proj(b + 1)
            for qc in range(NQC):
                attn_chunk(b, qc)
        em.flush()

    nc.compile()
    return nc


def prep_inputs(in_features, weight_q, weight_k, weight_v, weight_o, cfg: Cfg,
                n_cores=8):
    """Host-side shard/layout prep. Returns per-core input dicts."""
    import ml_dtypes
    mmnp = ml_dtypes.bfloat16
    B, S, D, T, KT = cfg.B, cfg.S, cfg.D, cfg.T, cfg.KT
    x = np.asarray(in_features, dtype=np.float32).reshape(T, D)
    # xT[p, kt, t] = x[t, kt*128 + p]
    xT = np.ascontiguousarray(
        x.T.reshape(KT, 128, T).transpose(1, 0, 2)).astype(mmnp)
    # additive causal mask for a 128x128 diagonal block: 0 keep, -60 kill
    maskneg = np.where(np.arange(128)[:, None] > np.arange(128)[None, :],
                       np.float32(-60.0), np.float32(0.0)).astype(mmnp)
    wq = np.asarray(weight_q, dtype=np.float32) * (1.0 / np.sqrt(cfg.HD))
    wk = np.asarray(weight_k, dtype=np.float32)
    wv = np.asarray(weight_v, dtype=np.float32)
    wo = np.asarray(weight_o, dtype=np.float32)

    def wslice(w, c):
        # [128, KT, 128]: ws[p, kt, m] = w[c*128 + m, kt*128 + p]
        ws = w[c * 128:(c + 1) * 128, :]                  # [128, D]
        return np.ascontiguousarray(
            ws.T.reshape(KT, 128, 128).transpose(1, 0, 2)).astype(mmnp)

    in_maps = []
    for c in range(n_cores):
        in_maps.append({
            "xT": xT,
            "wq": wslice(wq, c),
            "wk": wslice(wk, c),
            "wv": wslice(wv, c),
            "wo": np.ascontiguousarray(
                wo[:, c * 128:(c + 1) * 128].T).astype(mmnp),
            "maskneg": maskneg,
            "ident": np.eye(128, dtype=mmnp),
        })
    return in_maps


_CACHE = {}


def _get_program(cfg: Cfg):
    key = (cfg.B, cfg.S, cfg.D, cfg.TCH, cfg.QCH, cfg.mm_dt)
    if key not in _CACHE:
        _CACHE[key] = build_program(cfg)
    return _CACHE[key]


def run(inputs, cfg: Cfg, trace=False, trace_kwargs=None):
    import time
    from concourse.bass_utils import run_bass_kernel_spmd
    nc = _get_program(cfg)
    in_maps = prep_inputs(**inputs, cfg=cfg)
    last = None
    for attempt in range(3):
        try:
            res = run_bass_kernel_spmd(
                nc, in_maps, core_ids=list(range(8)), trace=trace,
                **(trace_kwargs or {}))
            break
        except Exception as e:  # transient NRT device wedges happen
            last = e
            time.sleep(10)
    else:
        raise last
    parts = [np.asarray(r["out_p"], dtype=np.float32) for r in res.results]
    out = np.sum(np.stack(parts, 0), axis=0)
    return out.astype(np.float32).reshape(cfg.B, cfg.S, cfg.D), res


def kernel(in_features, weight_q, weight_k, weight_v, weight_o):
    cfg = Cfg()
    out, _ = run(dict(in_features=in_features, weight_q=weight_q,
                      weight_k=weight_k, weight_v=weight_v,
                      weight_o=weight_o), cfg)
    return out
